# revision 64
# baseline (speedup 1.0000x reference)
# Trainium2 Bass kernel for nn_MultiHeadAttentionPure (B=2, S=1024, F=1024, H=16).
#
# The reference splits q/k/v into 64 feature-chunks of 16 ("groups"), runs
# causal attention independently per (group, batch) pair -- 128 independent
# [1024,16] attention problems -- then applies a (buggy-but-faithful) torch
# reshape that scrambles (group, batch, seq) into the [B,S,F] tensor fed to
# the output linear layer.
#
# Sharding: core c = b2*4 + q (q = s2 block of 256) needs exactly the 16
# groups {j : j%4 == 2*b2 + q//2} at input batch b = q%2 -- a perfect
# partition of the 128 (group, batch) pairs across 8 cores with zero
# cross-core traffic.  Each core computes its 16 attention groups, assembles
# its y^T tile ([1024 features, 256 rows]) on-chip, and runs the output
# linear for its 256 output rows.  Host slices inputs / concats outputs.
#
# On-device layout (per core, per group g), causal path:
#   - the 8 causal-diagonal score slabs (s1-widths 512/384/256/128 per
#     chunk) are matmul'd into ONE packed PSUM region [128, 2560] (5 banks)
#     so a single Exp activation covers all of them (ACT per-instr overhead
#     dominates otherwise); the 4 full slabs of chunk c=1 use per-tile
#     [128,512] PSUM tiles.
#   - causal triangles: gpsimd (Pool) adds a -1e9 triangle onto the 8
#     diagonal 128x128 sub-blocks of the packed region before Exp.
#   - x^T accumulation: va-aug stationaries [128,34] ([va(16)+ones | 0] for
#     chunk 0, [0 | va+ones] for chunk 1) accumulate both chunks into one
#     PSUM bank xt [34, 512]; row 16/33 = softmax denominator.
#   - normalize: DVE reciprocal of the denom row, Pool partition-broadcast
#     to 16 partitions, DVE multiplies (m-deinterleaved into xs [16,4,128]).
#   - scatter: ONE dma per (g, chunk): xs [16,4,128] -> yt partitions
#     64*(g%2)+4h+m (h-major order matches the linearized source).  W_out is
#     permuted on the host to match this feature order, so the output linear
#     is a plain fp16 matmul sweep over yt x wt + bias.
import numpy as np

B, S, F, H = 2, 1024, 1024, 16
NG = 16          # groups per core
P = 128
NCORES = 8
FP8 = True       # q/k in float8_e4m3 + DoubleRow matmuls (2x PE score rate)

# The 12 causal score slabs of a group (s1-chunks of 512, s2-tiles of 128)
# are packed into FIVE [128, <=1024] PSUM tiles ("waves"), each covered by a
# single Exp.  Entries: (chunk c, s2-tile t, a1 = xt column offset, width,
# packed column offset within the wave, tri).  Slab pieces never cross a
# PSUM bank (512-col) boundary.  tri=True slabs start with a 128x128 causal
# triangle at their packed offset.
WAVES = [
    [(1, 0, 0, 512, 0, False), (1, 1, 0, 512, 512, False)],
    [(1, 2, 0, 512, 0, False), (1, 3, 0, 512, 512, False)],
    [(0, 0, 0, 512, 0, True), (1, 4, 0, 512, 512, True)],
    [(0, 1, 128, 384, 0, True), (0, 3, 384, 128, 384, True),
     (0, 2, 256, 256, 512, True), (1, 6, 256, 256, 768, True)],
    [(1, 5, 128, 384, 0, True), (1, 7, 384, 128, 384, True)],
]


def _core_groups(c):
    b2, qq = c // 4, c % 4
    b = qq % 2
    jmod = 2 * b2 + qq // 2
    js = [4 * h2 + jmod for h2 in range(NG)]
    return b2, qq, b, js


def _perm_feature(p, blk):
    """Logical yt feature stored at partition p (0..127), block blk (0..7).
    p = 64*g0 + 4*h + m  ->  f = 128*blk + 64*g0 + 16*m + h."""
    g0, m, h = p // 64, p % 4, (p % 64) // 4
    return 128 * blk + 64 * g0 + 16 * m + h


def _build(causal: bool, n_iter: int = 1):
    import concourse.bass as bass
    import concourse.mybir as mybir
    from concourse import bacc, tile

    F32 = mybir.dt.float32
    F16 = mybir.dt.float16
    AF = mybir.ActivationFunctionType
    ADD = mybir.AluOpType.add
    MUL = mybir.AluOpType.mult

    F8 = mybir.dt.float8e4
    QDT = F8 if FP8 else F16
    DR = mybir.MatmulPerfMode.DoubleRow if FP8 else None

    nc = bacc.Bacc("TRN2", target_bir_lowering=False, debug=False)
    qkt = nc.declare_dram_parameter("qkt", [P, (32 if FP8 else 16) * S], QDT,
                                    isOutput=False)
    va = nc.declare_dram_parameter("va", [P, 8 * 528], F16, isOutput=False)
    wt = nc.declare_dram_parameter("wt", [P, (F // P) * F], F16, isOutput=False)
    # trid[:, 0:128] = -1e4 * [k < p] (strict lower triangle, stationary),
    # trid[:, 128:256] = identity.  tri-mask = trid[:, :128].T @ trid[:, 128:]
    trid = nc.declare_dram_parameter("trid", [P, 2 * P], F16, isOutput=False)
    bb = nc.declare_dram_parameter("bb", [P, F], F32, isOutput=False)
    out = nc.declare_dram_parameter("o", [256, F], F32, isOutput=True)

    NT = S // P           # 8 s2 tiles
    NC_ = 2               # s1 chunks of 512

    import contextlib
    with tile.TileContext(nc) as tc:
        loop_ctx = tc.For_i(0, n_iter, 1, hint_engines=(
            mybir.EngineType.PE, mybir.EngineType.DVE, mybir.EngineType.Activation,
            mybir.EngineType.SP, mybir.EngineType.Pool,
        )) if n_iter > 1 else contextlib.nullcontext()
        with loop_ctx, \
             tc.tile_pool(name="cst", bufs=1) as cst, \
             tc.tile_pool(name="exw", bufs=4) as exw, \
             tc.tile_pool(name="wk", bufs=3) as wkp, \
             tc.tile_pool(name="yt", bufs=1) as ytp, \
             tc.tile_pool(name="pk", bufs=3, space="PSUM") as pkp, \
             tc.tile_pool(name="xps", bufs=2, space="PSUM") as xps:

            qkt_sb = cst.tile([P, 16, 2, S] if FP8 else [P, 16, S], QDT)
            va_sb = cst.tile([P, NT, 528], F16)
            wt_sb = cst.tile([P, F // P, F], F16)
            trid_sb = cst.tile([P, 2 * P], F16)
            bb_sb = cst.tile([P, F], F32)
            nc.sync.dma_start(qkt_sb[:], qkt[:])
            nc.sync.dma_start(va_sb[:], va[:])
            nc.sync.dma_start(wt_sb[:], wt[:])
            nc.sync.dma_start(trid_sb[:], trid[:])
            nc.sync.dma_start(bb_sb[:], bb[:])

            yt_sb = ytp.tile([P, F // P, 256], F16)

            if causal:
                waves = WAVES
            else:
                # non-causal: 16 full slabs, 512-col pieces, waves of 2
                slabs = [(c, t, 0, 512, 512 * (i % 2), False)
                         for i, (c, t) in enumerate(
                             (c, t) for c in range(NC_) for t in range(NT))]
                waves = [slabs[i:i + 2] for i in range(0, 16, 2)]
            NW = len(waves)
            wave_list = [(g, wi) for g in range(NG) for wi in range(NW)]
            n_mm = sum(len(wv) for wv in waves)

            if FP8:
                # DoubleRow: [8 partitions, 2, cols]; head row r = p + 8*i
                def q_l(g, sl):
                    return qkt_sb[64 * (g % 2):64 * (g % 2) + 8, g // 2, :, sl]

                def k_l(g, sl):
                    return qkt_sb[64 * (g % 2):64 * (g % 2) + 8,
                                  8 + g // 2, :, sl]
            else:
                def q_l(g, sl):
                    return qkt_sb[64 * (g % 2):64 * (g % 2) + 32, g // 2, sl]

                def k_l(g, sl):
                    return qkt_sb[64 * (g % 2):64 * (g % 2) + 32,
                                  8 + g // 2, sl]

            def emit_scores(g, wi):
                """Score matmuls (with matmul'd causal tri-mask pre-bias) +
                one exp for wave wi of g."""
                wv = waves[wi]
                wlen = max(off + w for _, _, _, w, off, _ in wv)
                pkt = pkp.tile([P, 1024], F32, tag="pk", name=f"pk_{g}_{wi}")
                for c, t, a1, w, off, tri in wv:
                    if tri:
                        # causal triangle as a matmul: PSUM <- -1e4*[j<p],
                        # then the score matmul accumulates on top.
                        nc.tensor.matmul(
                            pkt[:, off:off + P], trid_sb[:, 0:P],
                            trid_sb[:, P:2 * P], start=True, stop=False)
                        nc.tensor.matmul(
                            pkt[:, off:off + P],
                            k_l(g, slice(t * P, (t + 1) * P)),
                            q_l(g, slice(512 * c + a1, 512 * c + a1 + P)),
                            start=False, stop=True, perf_mode=DR)
                        if w > P:
                            nc.tensor.matmul(
                                pkt[:, off + P:off + w],
                                k_l(g, slice(t * P, (t + 1) * P)),
                                q_l(g, slice(512 * c + a1 + P, 512 * (c + 1))),
                                start=True, stop=True, perf_mode=DR)
                    else:
                        nc.tensor.matmul(
                            pkt[:, off:off + w],
                            k_l(g, slice(t * P, (t + 1) * P)),
                            q_l(g, slice(512 * c + a1, 512 * (c + 1))),
                            start=True, stop=True, perf_mode=DR)
                ew = exw.tile([P, 1024], F16, tag="ew", name=f"ew_{g}_{wi}")
                nc.scalar.activation(ew[:, 0:wlen], pkt[:, 0:wlen], AF.Exp)
                return ew

            xt_of = {}
            # per-chunk xmm counts for start/stop flags
            n_mm_c = [sum(1 for wv in waves for s in wv if s[0] == c)
                      for c in range(NC_)]

            def emit_xmms(g, wi, ew):
                """x^T accumulation for wave wi of group g (lag pipelined).
                The 33-col stationary is [features(16) | zeros(16) | ones]:
                chunk 0 lands at xt partitions [0:33] (denominator row 32),
                chunk 1 at [64:97] (denominator row 96) -- feature reads at
                base 0/64 and denom reads at base 32/96 keep every engine AP
                32-partition aligned."""
                if wi == 0:
                    xt_of[g] = [xps.tile([P, 512], F32, tag="xt",
                                         name=f"xt_{g}"), [0] * NC_]
                xt, cnt = xt_of[g]
                for c, t, a1, w, off, tri in waves[wi]:
                    fo = 64 * c
                    nc.tensor.matmul(
                        xt[fo:fo + 33, a1:a1 + w] if causal
                        else xt[fo:fo + 33, 0:512],
                        va_sb[:, t, 33 * g:33 * g + 33], ew[:, off:off + w],
                        start=(cnt[c] == 0), stop=(cnt[c] == n_mm_c[c] - 1))
                    cnt[c] += 1

            def emit_normalize(g):
                """softmax-normalize + scatter both chunks of group g."""
                xt, _ = xt_of.pop(g)
                for c in range(NC_):
                    fo = 64 * c     # feature/denom partition offset in xt
                    recip = wkp.tile([1, 512], F16, tag="recip",
                                     name=f"rc_{g}_{c}")
                    with nc.allow_low_precision(reason="fp16 softmax recip"):
                        nc.vector.reciprocal(recip[:], xt[fo + 32:fo + 33, :])
                    recipb = wkp.tile([16, 512], F16, tag="recipb",
                                      name=f"rb_{g}_{c}")
                    nc.gpsimd.partition_broadcast(recipb[:], recip[:])
                    xs = wkp.tile([16, 4, 128], F16, tag="xs",
                                  name=f"xs_{g}_{c}")
                    for m in range(4):
                        nc.vector.tensor_tensor(
                            out=xs[:, m, :], in0=xt[fo:fo + 16, m:512:4],
                            in1=recipb[:, m:512:4], op=MUL)
                    g0 = g % 2
                    nc.sync.dma_start(
                        out=yt_sb[64 * g0:64 * g0 + 64, g // 2,
                                  128 * c:128 * (c + 1)],
                        in_=xs[:])

            # software pipeline: scores(w) || xmms(w-LAG), normalize at
            # group boundaries
            LAG = 3
            ew_of = {}
            for w, (g, wi) in enumerate(wave_list):
                ew_of[(g, wi)] = emit_scores(g, wi)
                if w >= LAG:
                    gp, wip = wave_list[w - LAG]
                    emit_xmms(gp, wip, ew_of.pop((gp, wip)))
                    if wip == NW - 1:
                        emit_normalize(gp)
            for w in range(len(wave_list) - LAG, len(wave_list)):
                gp, wip = wave_list[w]
                emit_xmms(gp, wip, ew_of.pop((gp, wip)))
                if wip == NW - 1:
                    emit_normalize(gp)

            # output linear: out[r, o] = sum_f yT[f, r] * wt[f, o] + b[o]
            for r2 in range(2):
                for oc in range(2):
                    ps_t = xps.tile([P, 512], F32, tag="xt")
                    ps = ps_t[:]
                    for ft_i in range(F // P):
                        nc.tensor.matmul(
                            ps, yt_sb[:, ft_i, r2 * P:(r2 + 1) * P],
                            wt_sb[:, ft_i, oc * 512:(oc + 1) * 512],
                            start=(ft_i == 0), stop=(ft_i == F // P - 1))
                    ot = wkp.tile([P, 512], F32, tag="ot")
                    nc.vector.tensor_tensor(
                        out=ot[:], in0=ps, in1=bb_sb[:, oc * 512:(oc + 1) * 512],
                        op=ADD)
                    nc.sync.dma_start(
                        out=out[r2 * P:(r2 + 1) * P, oc * 512:(oc + 1) * 512],
                        in_=ot[:])
    nc.compile()
    return nc


_NC_CACHE = {}


def _get_nc(causal: bool, n_iter: int = 1):
    key = (causal, n_iter)
    if key not in _NC_CACHE:
        _NC_CACHE[key] = _build(causal, n_iter)
    return _NC_CACHE[key]


def _shard_inputs(q, k, v, W_out, b_out):
    """Build the 8 per-core input maps."""
    wtT = np.ascontiguousarray(W_out.T).astype(np.float32)   # [f_in, o]
    wtp = np.empty((P, F // P, F), np.float16)
    for blk in range(F // P):
        for p in range(P):
            wtp[p, blk, :] = wtT[_perm_feature(p, blk), :]
    wtp = wtp.reshape(P, (F // P) * F)

    ki, pi = np.mgrid[0:P, 0:P]
    tridv = np.zeros((P, 2 * P), np.float16)
    tridv[:, 0:P] = np.where(ki < pi, -1e4, 0.0)      # strict lower tri
    tridv[:, P:2 * P] = np.where(ki == pi, 1.0, 0.0)  # identity
    bbv = np.broadcast_to(b_out.astype(np.float32), (P, F)).copy()

    if FP8:
        import concourse.mybir as mybir
        qdt = mybir.dt.np(mybir.dt.float8e4)

    in_maps = []
    for core in range(NCORES):
        _, _, b, js = _core_groups(core)
        cols = np.stack([j * H + np.arange(H) for j in js])    # [16, 16]
        if FP8:
            qkt = np.zeros((P, 16, 2, S), qdt)
        else:
            qkt = np.zeros((P, 16, S), np.float16)
        vav = np.zeros((P, NT_ := S // P, 528), np.float16)
        for g in range(NG):
            po, pb = 64 * (g % 2), g // 2
            qg = (0.25 * q[b][:, cols[g]].T)                   # [16, S]
            kg = k[b][:, cols[g]].T
            if FP8:
                # DoubleRow packing: head row r = p + 8*i
                qkt[po:po + 8, pb, 0, :] = qg[0:8].astype(qdt)
                qkt[po:po + 8, pb, 1, :] = qg[8:16].astype(qdt)
                qkt[po:po + 8, 8 + pb, 0, :] = kg[0:8].astype(qdt)
                qkt[po:po + 8, 8 + pb, 1, :] = kg[8:16].astype(qdt)
            else:
                qkt[po:po + 16, pb, :] = qg.astype(np.float16)
                qkt[po:po + 16, 8 + pb, :] = kg.astype(np.float16)
            vg = v[b][:, cols[g]].astype(np.float16)           # [S, 16]
            vg = vg.reshape(NT_, P, H).transpose(1, 0, 2)      # [P, NT, 16]
            vav[:, :, 33 * g:33 * g + 16] = vg
            vav[:, :, 33 * g + 32] = 1.0
        in_maps.append({
            "qkt": qkt.reshape(P, (32 if FP8 else 16) * S),
            "va": vav.reshape(P, (S // P) * 528),
            "wt": wtp,
            "trid": tridv,
            "bb": bbv,
        })
    return in_maps


def _unshard(outs):
    full = np.empty((B, S, F), np.float32)
    for c in range(NCORES):
        b2, qq, _, _ = _core_groups(c)
        full[b2, 256 * qq:256 * (qq + 1), :] = outs[c]
    return full


def _numpy_core(in_map, causal=True):
    """Numpy emulation of the device program (host-logic validation)."""
    qkt = in_map["qkt"].reshape(P, 16, S).astype(np.float32)
    va = in_map["va"].reshape(P, S // P, 528).astype(np.float32)
    wtm = in_map["wt"].reshape(P, F // P, F).astype(np.float32)
    bbv = in_map["bb"]
    ytv = np.zeros((P, F // P, 256), np.float32)
    for g in range(NG):
        po, pb = 64 * (g % 2), g // 2
        qt = qkt[po:po + 16, pb, :]       # [16, S], pre-scaled
        kt = qkt[po:po + 16, 8 + pb, :]
        sc = kt.T @ qt                    # [s2, s1]
        if causal:
            s2i, s1i = np.mgrid[0:S, 0:S]
            sc = np.where(s1i >= s2i, sc, -1e9)
        e = np.exp(sc).astype(np.float16).astype(np.float32)
        if causal:
            e = np.where(s1i >= s2i, e, 0.0).astype(np.float32)
        # x accumulation with augmented va (feature rows + ones row at 32)
        vg = np.zeros((S, 33), np.float32)
        for t in range(S // P):
            vg[t * P:(t + 1) * P, :] = va[:, t, 33 * g:33 * g + 33]
        xt = vg.T @ e                     # [33, s1]
        recip = (1.0 / xt[32]).astype(np.float16).astype(np.float32)
        xs = (xt[0:16] * recip[None, :]).astype(np.float16).astype(np.float32)
        g0 = g % 2
        for c in range(2):
            for h in range(16):
                for m in range(4):
                    p = 64 * g0 + 4 * h + m
                    ytv[p, g // 2, 128 * c:128 * (c + 1)] = \
                        xs[h, 512 * c + m:512 * (c + 1):4]
    yt2 = ytv.reshape(P, (F // P) * 256)
    o = np.zeros((256, F), np.float32)
    for r2 in range(2):
        for oc in range(2):
            acc = np.zeros((P, 512), np.float32)
            for ft_i in range(F // P):
                acc += ytv[:, ft_i, r2 * P:(r2 + 1) * P].T @ \
                    wtm[:, ft_i, oc * 512:(oc + 1) * 512]
            o[r2 * P:(r2 + 1) * P, oc * 512:(oc + 1) * 512] = \
                acc + bbv[0][None, oc * 512:(oc + 1) * 512]
    return o.astype(np.float32)


def kernel(q, k, v, W_out, b_out, apply_mask, _mock=False):
    q = np.asarray(q, np.float32)
    k = np.asarray(k, np.float32)
    v = np.asarray(v, np.float32)
    W_out = np.asarray(W_out, np.float32)
    b_out = np.asarray(b_out, np.float32)
    causal = bool(int(np.asarray(apply_mask)))
    in_maps = _shard_inputs(q, k, v, W_out, b_out)
    if _mock:
        outs = [_numpy_core(m, causal) for m in in_maps]
        return _unshard(outs)
    from concourse.bass_utils import run_bass_kernel_spmd
    nc = _get_nc(causal)
    res = run_bass_kernel_spmd(nc, in_maps, core_ids=list(range(NCORES)))
    return _unshard([r["o"] for r in res.results])


# revision 67
# speedup vs baseline: 1.3684x; 1.3684x over previous
# Trainium2 Bass kernel for nn_MultiHeadAttentionPure (B=2, S=1024, F=1024, H=16).
#
# The reference splits q/k/v into 64 feature-chunks of 16 ("groups"), runs
# causal attention independently per (group, batch) pair -- 128 independent
# [1024,16] attention problems -- then applies a (buggy-but-faithful) torch
# reshape that scrambles (group, batch, seq) into the [B,S,F] tensor fed to
# the output linear layer.
#
# Sharding: core c = b2*4 + q (q = s2 block of 256) needs exactly the 16
# groups {j : j%4 == 2*b2 + q//2} at input batch b = q%2 -- a perfect
# partition of the 128 (group, batch) pairs across 8 cores with zero
# cross-core traffic.  Each core computes its 16 attention groups, assembles
# its y^T tile ([1024 features, 256 rows]) on-chip, and runs the output
# linear for its 256 output rows.  Host slices inputs / concats outputs.
#
# On-device layout (per core, per group g), causal path:
#   - the 8 causal-diagonal score slabs (s1-widths 512/384/256/128 per
#     chunk) are matmul'd into ONE packed PSUM region [128, 2560] (5 banks)
#     so a single Exp activation covers all of them (ACT per-instr overhead
#     dominates otherwise); the 4 full slabs of chunk c=1 use per-tile
#     [128,512] PSUM tiles.
#   - causal triangles: gpsimd (Pool) adds a -1e9 triangle onto the 8
#     diagonal 128x128 sub-blocks of the packed region before Exp.
#   - x^T accumulation: va-aug stationaries [128,34] ([va(16)+ones | 0] for
#     chunk 0, [0 | va+ones] for chunk 1) accumulate both chunks into one
#     PSUM bank xt [34, 512]; row 16/33 = softmax denominator.
#   - normalize: DVE reciprocal of the denom row, Pool partition-broadcast
#     to 16 partitions, DVE multiplies (m-deinterleaved into xs [16,4,128]).
#   - scatter: ONE dma per (g, chunk): xs [16,4,128] -> yt partitions
#     64*(g%2)+4h+m (h-major order matches the linearized source).  W_out is
#     permuted on the host to match this feature order, so the output linear
#     is a plain fp16 matmul sweep over yt x wt + bias.
import numpy as np

B, S, F, H = 2, 1024, 1024, 16
NG = 16          # groups per core
P = 128
NCORES = 8
# q/k in float8_e4m3 + DoubleRow matmuls (2x PE score rate).  Works, but
# end-to-end max-norm error is 1.9e-2 -- too close to the 2e-2 gate.
FP8 = False

# The 12 causal score slabs of a group (s1-chunks of 512, s2-tiles of 128)
# are packed into FIVE [128, <=1024] PSUM tiles ("waves"), each covered by a
# single Exp.  Entries: (chunk c, s2-tile t, a1 = xt column offset, width,
# packed column offset within the wave, tri).  Slab pieces never cross a
# PSUM bank (512-col) boundary.  tri=True slabs start with a 128x128 causal
# triangle at their packed offset.
WAVES = [
    [(1, 0, 0, 512, 0, False), (1, 1, 0, 512, 512, False)],
    [(1, 2, 0, 512, 0, False), (1, 3, 0, 512, 512, False)],
    [(0, 0, 0, 512, 0, True), (1, 4, 0, 512, 512, True)],
    [(0, 1, 128, 384, 0, True), (0, 3, 384, 128, 384, True),
     (0, 2, 256, 256, 512, True), (1, 6, 256, 256, 768, True)],
    [(1, 5, 128, 384, 0, True), (1, 7, 384, 128, 384, True)],
]


def _core_groups(c):
    b2, qq = c // 4, c % 4
    b = qq % 2
    jmod = 2 * b2 + qq // 2
    js = [4 * h2 + jmod for h2 in range(NG)]
    return b2, qq, b, js


def _perm_feature(p, blk):
    """Logical yt feature stored at partition p (0..127), block blk (0..7).
    p = 64*g0 + 4*h + m  ->  f = 128*blk + 64*g0 + 16*m + h."""
    g0, m, h = p // 64, p % 4, (p % 64) // 4
    return 128 * blk + 64 * g0 + 16 * m + h


def _build(causal: bool, n_iter: int = 1):
    import concourse.bass as bass
    import concourse.mybir as mybir
    from concourse import bacc, tile

    F32 = mybir.dt.float32
    F16 = mybir.dt.float16
    AF = mybir.ActivationFunctionType
    ADD = mybir.AluOpType.add
    MUL = mybir.AluOpType.mult

    F8 = mybir.dt.float8e4
    QDT = F8 if FP8 else F16
    DR = mybir.MatmulPerfMode.DoubleRow if FP8 else None

    nc = bacc.Bacc("TRN2", target_bir_lowering=False, debug=False)
    qkt = nc.declare_dram_parameter("qkt", [P, (32 if FP8 else 16) * S], QDT,
                                    isOutput=False)
    va = nc.declare_dram_parameter("va", [P, 8 * 528], F16, isOutput=False)
    wt = nc.declare_dram_parameter("wt", [P, (F // P) * F], F16, isOutput=False)
    # trid[:, 0:128] = -1e4 * [k < p] (strict lower triangle, stationary),
    # trid[:, 128:256] = identity.  tri-mask = trid[:, :128].T @ trid[:, 128:]
    trid = nc.declare_dram_parameter("trid", [P, 2 * P], F16, isOutput=False)
    bb = nc.declare_dram_parameter("bb", [P, F], F32, isOutput=False)
    out = nc.declare_dram_parameter("o", [256, F], F32, isOutput=True)

    NT = S // P           # 8 s2 tiles
    NC_ = 2               # s1 chunks of 512

    import contextlib
    with tile.TileContext(nc) as tc:
        loop_ctx = tc.For_i(0, n_iter, 1, hint_engines=(
            mybir.EngineType.PE, mybir.EngineType.DVE, mybir.EngineType.Activation,
            mybir.EngineType.SP, mybir.EngineType.Pool,
        )) if n_iter > 1 else contextlib.nullcontext()
        with loop_ctx, \
             tc.tile_pool(name="cst", bufs=1) as cst, \
             tc.tile_pool(name="exw", bufs=4) as exw, \
             tc.tile_pool(name="wk", bufs=3) as wkp, \
             tc.tile_pool(name="yt", bufs=1) as ytp, \
             tc.tile_pool(name="pk", bufs=3, space="PSUM") as pkp, \
             tc.tile_pool(name="xps", bufs=2, space="PSUM") as xps:

            qkt_sb = cst.tile([P, 16, 2, S] if FP8 else [P, 16, S], QDT)
            va_sb = cst.tile([P, NT, 528], F16)
            wt_sb = cst.tile([P, F // P, F], F16)
            trid_sb = cst.tile([P, 2 * P], F16)
            bb_sb = cst.tile([P, F], F32)
            nc.sync.dma_start(qkt_sb[:], qkt[:])
            nc.sync.dma_start(va_sb[:], va[:])
            nc.sync.dma_start(wt_sb[:], wt[:])
            nc.sync.dma_start(trid_sb[:], trid[:])
            nc.sync.dma_start(bb_sb[:], bb[:])

            yt_sb = ytp.tile([P, F // P, 256], F16)

            if causal:
                waves = WAVES
            else:
                # non-causal: 16 full slabs, 512-col pieces, waves of 2
                slabs = [(c, t, 0, 512, 512 * (i % 2), False)
                         for i, (c, t) in enumerate(
                             (c, t) for c in range(NC_) for t in range(NT))]
                waves = [slabs[i:i + 2] for i in range(0, 16, 2)]
            NW = len(waves)
            wave_list = [(g, wi) for g in range(NG) for wi in range(NW)]
            n_mm = sum(len(wv) for wv in waves)

            if FP8:
                # DoubleRow: [8 partitions, 2, cols]; head row r = p + 8*i
                def q_l(g, sl):
                    return qkt_sb[64 * (g % 2):64 * (g % 2) + 8, g // 2, :, sl]

                def k_l(g, sl):
                    return qkt_sb[64 * (g % 2):64 * (g % 2) + 8,
                                  8 + g // 2, :, sl]
            else:
                def q_l(g, sl):
                    return qkt_sb[64 * (g % 2):64 * (g % 2) + 32, g // 2, sl]

                def k_l(g, sl):
                    return qkt_sb[64 * (g % 2):64 * (g % 2) + 32,
                                  8 + g // 2, sl]

            def emit_scores(g, wi):
                """Score matmuls (with matmul'd causal tri-mask pre-bias) +
                one exp for wave wi of g."""
                wv = waves[wi]
                wlen = max(off + w for _, _, _, w, off, _ in wv)
                pkt = pkp.tile([P, 1024], F32, tag="pk", name=f"pk_{g}_{wi}")
                for c, t, a1, w, off, tri in wv:
                    if tri:
                        # causal triangle as a matmul: PSUM <- -1e4*[j<p],
                        # then the score matmul accumulates on top.
                        nc.tensor.matmul(
                            pkt[:, off:off + P], trid_sb[:, 0:P],
                            trid_sb[:, P:2 * P], start=True, stop=False)
                        nc.tensor.matmul(
                            pkt[:, off:off + P],
                            k_l(g, slice(t * P, (t + 1) * P)),
                            q_l(g, slice(512 * c + a1, 512 * c + a1 + P)),
                            start=False, stop=True, perf_mode=DR)
                        if w > P:
                            nc.tensor.matmul(
                                pkt[:, off + P:off + w],
                                k_l(g, slice(t * P, (t + 1) * P)),
                                q_l(g, slice(512 * c + a1 + P, 512 * (c + 1))),
                                start=True, stop=True, perf_mode=DR)
                    else:
                        nc.tensor.matmul(
                            pkt[:, off:off + w],
                            k_l(g, slice(t * P, (t + 1) * P)),
                            q_l(g, slice(512 * c + a1, 512 * (c + 1))),
                            start=True, stop=True, perf_mode=DR)
                ew = exw.tile([P, 1024], F16, tag="ew", name=f"ew_{g}_{wi}")
                nc.scalar.activation(ew[:, 0:wlen], pkt[:, 0:wlen], AF.Exp)
                return ew

            xt_of = {}
            # per-chunk xmm counts for start/stop flags
            n_mm_c = [sum(1 for wv in waves for s in wv if s[0] == c)
                      for c in range(NC_)]

            def emit_xmms(g, wi, ew):
                """x^T accumulation for wave wi of group g (lag pipelined).
                The 33-col stationary is [features(16) | zeros(16) | ones]:
                chunk 0 lands at xt partitions [0:33] (denominator row 32),
                chunk 1 at [64:97] (denominator row 96) -- feature reads at
                base 0/64 and denom reads at base 32/96 keep every engine AP
                32-partition aligned."""
                if wi == 0:
                    xt_of[g] = [xps.tile([P, 512], F32, tag="xt",
                                         name=f"xt_{g}"), [0] * NC_]
                xt, cnt = xt_of[g]
                for c, t, a1, w, off, tri in waves[wi]:
                    fo = 64 * c
                    nc.tensor.matmul(
                        xt[fo:fo + 33, a1:a1 + w] if causal
                        else xt[fo:fo + 33, 0:512],
                        va_sb[:, t, 33 * g:33 * g + 33], ew[:, off:off + w],
                        start=(cnt[c] == 0), stop=(cnt[c] == n_mm_c[c] - 1))
                    cnt[c] += 1

            def emit_normalize(g):
                """softmax-normalize + scatter both chunks of group g."""
                xt, _ = xt_of.pop(g)
                for c in range(NC_):
                    fo = 64 * c     # feature/denom partition offset in xt
                    recip = wkp.tile([1, 512], F16, tag="recip",
                                     name=f"rc_{g}_{c}")
                    with nc.allow_low_precision(reason="fp16 softmax recip"):
                        nc.vector.reciprocal(recip[:], xt[fo + 32:fo + 33, :])
                    recipb = wkp.tile([16, 512], F16, tag="recipb",
                                      name=f"rb_{g}_{c}")
                    nc.gpsimd.partition_broadcast(recipb[:], recip[:])
                    xs = wkp.tile([16, 4, 128], F16, tag="xs",
                                  name=f"xs_{g}_{c}")
                    # one strided op m-deinterleaves: out (m, r) <- col 4r+m
                    xt_v = xt[fo:fo + 16, 0:4:1, 0:512:4]
                    rb_v = recipb[:, 0:4:1, 0:512:4]
                    nc.vector.tensor_tensor(
                        out=xs[:], in0=xt_v, in1=rb_v, op=MUL)
                    g0 = g % 2
                    nc.sync.dma_start(
                        out=yt_sb[64 * g0:64 * g0 + 64, g // 2,
                                  128 * c:128 * (c + 1)],
                        in_=xs[:])

            # software pipeline: scores(w) || xmms(w-LAG), normalize at
            # group boundaries
            LAG = 3
            ew_of = {}
            for w, (g, wi) in enumerate(wave_list):
                ew_of[(g, wi)] = emit_scores(g, wi)
                if w >= LAG:
                    gp, wip = wave_list[w - LAG]
                    emit_xmms(gp, wip, ew_of.pop((gp, wip)))
                    if wip == NW - 1:
                        emit_normalize(gp)
            for w in range(len(wave_list) - LAG, len(wave_list)):
                gp, wip = wave_list[w]
                emit_xmms(gp, wip, ew_of.pop((gp, wip)))
                if wip == NW - 1:
                    emit_normalize(gp)

            # output linear: out[r, o] = sum_f yT[f, r] * wt[f, o] + b[o]
            for r2 in range(2):
                for oc in range(2):
                    ps_t = xps.tile([P, 512], F32, tag="xt")
                    ps = ps_t[:]
                    for ft_i in range(F // P):
                        nc.tensor.matmul(
                            ps, yt_sb[:, ft_i, r2 * P:(r2 + 1) * P],
                            wt_sb[:, ft_i, oc * 512:(oc + 1) * 512],
                            start=(ft_i == 0), stop=(ft_i == F // P - 1))
                    ot = wkp.tile([P, 512], F32, tag="ot")
                    nc.vector.tensor_tensor(
                        out=ot[:], in0=ps, in1=bb_sb[:, oc * 512:(oc + 1) * 512],
                        op=ADD)
                    nc.sync.dma_start(
                        out=out[r2 * P:(r2 + 1) * P, oc * 512:(oc + 1) * 512],
                        in_=ot[:])
    nc.compile()
    return nc


_NC_CACHE = {}


def _get_nc(causal: bool, n_iter: int = 1):
    key = (causal, n_iter)
    if key not in _NC_CACHE:
        _NC_CACHE[key] = _build(causal, n_iter)
    return _NC_CACHE[key]


def _shard_inputs(q, k, v, W_out, b_out):
    """Build the 8 per-core input maps."""
    wtT = np.ascontiguousarray(W_out.T).astype(np.float32)   # [f_in, o]
    wtp = np.empty((P, F // P, F), np.float16)
    for blk in range(F // P):
        for p in range(P):
            wtp[p, blk, :] = wtT[_perm_feature(p, blk), :]
    wtp = wtp.reshape(P, (F // P) * F)

    ki, pi = np.mgrid[0:P, 0:P]
    tridv = np.zeros((P, 2 * P), np.float16)
    tridv[:, 0:P] = np.where(ki < pi, -1e4, 0.0)      # strict lower tri
    tridv[:, P:2 * P] = np.where(ki == pi, 1.0, 0.0)  # identity
    bbv = np.broadcast_to(b_out.astype(np.float32), (P, F)).copy()

    if FP8:
        import concourse.mybir as mybir
        qdt = mybir.dt.np(mybir.dt.float8e4)

    in_maps = []
    for core in range(NCORES):
        _, _, b, js = _core_groups(core)
        cols = np.stack([j * H + np.arange(H) for j in js])    # [16, 16]
        if FP8:
            qkt = np.zeros((P, 16, 2, S), qdt)
        else:
            qkt = np.zeros((P, 16, S), np.float16)
        vav = np.zeros((P, NT_ := S // P, 528), np.float16)
        for g in range(NG):
            po, pb = 64 * (g % 2), g // 2
            qg = (0.25 * q[b][:, cols[g]].T)                   # [16, S]
            kg = k[b][:, cols[g]].T
            if FP8:
                # DoubleRow packing: head row r = p + 8*i
                qkt[po:po + 8, pb, 0, :] = qg[0:8].astype(qdt)
                qkt[po:po + 8, pb, 1, :] = qg[8:16].astype(qdt)
                qkt[po:po + 8, 8 + pb, 0, :] = kg[0:8].astype(qdt)
                qkt[po:po + 8, 8 + pb, 1, :] = kg[8:16].astype(qdt)
            else:
                qkt[po:po + 16, pb, :] = qg.astype(np.float16)
                qkt[po:po + 16, 8 + pb, :] = kg.astype(np.float16)
            vg = v[b][:, cols[g]].astype(np.float16)           # [S, 16]
            vg = vg.reshape(NT_, P, H).transpose(1, 0, 2)      # [P, NT, 16]
            vav[:, :, 33 * g:33 * g + 16] = vg
            vav[:, :, 33 * g + 32] = 1.0
        in_maps.append({
            "qkt": qkt.reshape(P, (32 if FP8 else 16) * S),
            "va": vav.reshape(P, (S // P) * 528),
            "wt": wtp,
            "trid": tridv,
            "bb": bbv,
        })
    return in_maps


def _unshard(outs):
    full = np.empty((B, S, F), np.float32)
    for c in range(NCORES):
        b2, qq, _, _ = _core_groups(c)
        full[b2, 256 * qq:256 * (qq + 1), :] = outs[c]
    return full


def _numpy_core(in_map, causal=True):
    """Numpy emulation of the device program (host-logic validation)."""
    if FP8:
        qkt = in_map["qkt"].reshape(P, 16, 2, S).astype(np.float32)
    else:
        qkt = in_map["qkt"].reshape(P, 16, S).astype(np.float32)
    va = in_map["va"].reshape(P, S // P, 528).astype(np.float32)
    wtm = in_map["wt"].reshape(P, F // P, F).astype(np.float32)
    bbv = in_map["bb"]
    ytv = np.zeros((P, F // P, 256), np.float32)
    for g in range(NG):
        po, pb = 64 * (g % 2), g // 2
        if FP8:
            qt = qkt[po:po + 8, pb].transpose(1, 0, 2).reshape(16, S)
            kt = qkt[po:po + 8, 8 + pb].transpose(1, 0, 2).reshape(16, S)
        else:
            qt = qkt[po:po + 16, pb, :]   # [16, S], pre-scaled
            kt = qkt[po:po + 16, 8 + pb, :]
        sc = kt.T @ qt                    # [s2, s1]
        if causal:
            s2i, s1i = np.mgrid[0:S, 0:S]
            sc = np.where(s1i >= s2i, sc, -1e9)
        e = np.exp(sc).astype(np.float16).astype(np.float32)
        if causal:
            e = np.where(s1i >= s2i, e, 0.0).astype(np.float32)
        # x accumulation with augmented va (feature rows + ones row at 32)
        vg = np.zeros((S, 33), np.float32)
        for t in range(S // P):
            vg[t * P:(t + 1) * P, :] = va[:, t, 33 * g:33 * g + 33]
        xt = vg.T @ e                     # [33, s1]
        recip = (1.0 / xt[32]).astype(np.float16).astype(np.float32)
        xs = (xt[0:16] * recip[None, :]).astype(np.float16).astype(np.float32)
        g0 = g % 2
        for c in range(2):
            for h in range(16):
                for m in range(4):
                    p = 64 * g0 + 4 * h + m
                    ytv[p, g // 2, 128 * c:128 * (c + 1)] = \
                        xs[h, 512 * c + m:512 * (c + 1):4]
    yt2 = ytv.reshape(P, (F // P) * 256)
    o = np.zeros((256, F), np.float32)
    for r2 in range(2):
        for oc in range(2):
            acc = np.zeros((P, 512), np.float32)
            for ft_i in range(F // P):
                acc += ytv[:, ft_i, r2 * P:(r2 + 1) * P].T @ \
                    wtm[:, ft_i, oc * 512:(oc + 1) * 512]
            o[r2 * P:(r2 + 1) * P, oc * 512:(oc + 1) * 512] = \
                acc + bbv[0][None, oc * 512:(oc + 1) * 512]
    return o.astype(np.float32)


def kernel(q, k, v, W_out, b_out, apply_mask, _mock=False):
    q = np.asarray(q, np.float32)
    k = np.asarray(k, np.float32)
    v = np.asarray(v, np.float32)
    W_out = np.asarray(W_out, np.float32)
    b_out = np.asarray(b_out, np.float32)
    causal = bool(int(np.asarray(apply_mask)))
    in_maps = _shard_inputs(q, k, v, W_out, b_out)
    if _mock:
        outs = [_numpy_core(m, causal) for m in in_maps]
        return _unshard(outs)
    from concourse.bass_utils import run_bass_kernel_spmd
    nc = _get_nc(causal)
    res = run_bass_kernel_spmd(nc, in_maps, core_ids=list(range(NCORES)))
    return _unshard([r["o"] for r in res.results])


# revision 71
# speedup vs baseline: 1.7357x; 1.2684x over previous
# Trainium2 Bass kernel for nn_MultiHeadAttentionPure (B=2, S=1024, F=1024, H=16).
#
# The reference splits q/k/v into 64 feature-chunks of 16 ("groups"), runs
# causal attention independently per (group, batch) pair -- 128 independent
# [1024,16] attention problems -- then applies a (buggy-but-faithful) torch
# reshape that scrambles (group, batch, seq) into the [B,S,F] tensor fed to
# the output linear layer.
#
# Sharding: core c = b2*4 + q (q = s2 block of 256) needs exactly the 16
# groups {j : j%4 == 2*b2 + q//2} at input batch b = q%2 -- a perfect
# partition of the 128 (group, batch) pairs across 8 cores with zero
# cross-core traffic.  Each core computes its 16 attention groups, assembles
# its y^T tile ([1024 features, 256 rows]) on-chip, and runs the output
# linear for its 256 output rows.  Host slices inputs / concats outputs.
#
# On-device layout (per core, per group g), causal path:
#   - the 8 causal-diagonal score slabs (s1-widths 512/384/256/128 per
#     chunk) are matmul'd into ONE packed PSUM region [128, 2560] (5 banks)
#     so a single Exp activation covers all of them (ACT per-instr overhead
#     dominates otherwise); the 4 full slabs of chunk c=1 use per-tile
#     [128,512] PSUM tiles.
#   - causal triangles: gpsimd (Pool) adds a -1e9 triangle onto the 8
#     diagonal 128x128 sub-blocks of the packed region before Exp.
#   - x^T accumulation: va-aug stationaries [128,34] ([va(16)+ones | 0] for
#     chunk 0, [0 | va+ones] for chunk 1) accumulate both chunks into one
#     PSUM bank xt [34, 512]; row 16/33 = softmax denominator.
#   - normalize: DVE reciprocal of the denom row, Pool partition-broadcast
#     to 16 partitions, DVE multiplies (m-deinterleaved into xs [16,4,128]).
#   - scatter: ONE dma per (g, chunk): xs [16,4,128] -> yt partitions
#     64*(g%2)+4h+m (h-major order matches the linearized source).  W_out is
#     permuted on the host to match this feature order, so the output linear
#     is a plain fp16 matmul sweep over yt x wt + bias.
import numpy as np

B, S, F, H = 2, 1024, 1024, 16
NG = 16          # groups per core
P = 128
NCORES = 8
# q/k in float8_e4m3 + DoubleRow matmuls (2x PE score rate).  Works, but
# end-to-end max-norm error is 1.9e-2 -- too close to the 2e-2 gate.
FP8 = False

# The 12 causal score slabs of a group (s1-chunks of 512, s2-tiles of 128)
# are packed into FIVE [128, <=1024] PSUM tiles ("waves"), each covered by a
# single Exp.  Entries: (chunk c, s2-tile t, a1 = xt column offset, width,
# packed column offset within the wave, tri).  Slab pieces never cross a
# PSUM bank (512-col) boundary.  tri=True slabs start with a 128x128 causal
# triangle at their packed offset.
WAVES = [
    [(1, 0, 0, 512, 0, False), (1, 1, 0, 512, 512, False)],
    [(1, 2, 0, 512, 0, False), (1, 3, 0, 512, 512, False)],
    [(0, 0, 0, 512, 0, True), (1, 4, 0, 512, 512, True)],
    [(0, 1, 128, 384, 0, True), (0, 3, 384, 128, 384, True),
     (0, 2, 256, 256, 512, True), (1, 6, 256, 256, 768, True)],
    [(1, 5, 128, 384, 0, True), (1, 7, 384, 128, 384, True)],
]


def _core_groups(c):
    b2, qq = c // 4, c % 4
    b = qq % 2
    jmod = 2 * b2 + qq // 2
    js = [4 * h2 + jmod for h2 in range(NG)]
    return b2, qq, b, js


def _perm_feature(p, blk):
    """Logical yt feature stored at partition p (0..127), block blk (0..7).
    p = 64*g0 + 4*h + m  ->  f = 128*blk + 64*g0 + 16*m + h."""
    g0, m, h = p // 64, p % 4, (p % 64) // 4
    return 128 * blk + 64 * g0 + 16 * m + h


def _build(causal: bool, n_iter: int = 1):
    import concourse.bass as bass
    import concourse.mybir as mybir
    from concourse import bacc, tile

    F32 = mybir.dt.float32
    F16 = mybir.dt.float16
    AF = mybir.ActivationFunctionType
    ADD = mybir.AluOpType.add
    MUL = mybir.AluOpType.mult

    F8 = mybir.dt.float8e4
    QDT = F8 if FP8 else F16
    DR = mybir.MatmulPerfMode.DoubleRow if FP8 else None

    nc = bacc.Bacc("TRN2", target_bir_lowering=False, debug=False)
    qkt = nc.declare_dram_parameter("qkt", [P, (32 if FP8 else 16) * S], QDT,
                                    isOutput=False)
    va = nc.declare_dram_parameter("va", [P, 8 * 528], F16, isOutput=False)
    wt = nc.declare_dram_parameter("wt", [P, (F // P) * F], F16, isOutput=False)
    # trid[:, 0:128] = -1e4 * [k < p] (strict lower triangle, stationary),
    # trid[:, 128:256] = identity.  tri-mask = trid[:, :128].T @ trid[:, 128:]
    trid = nc.declare_dram_parameter("trid", [P, 2 * P], F16, isOutput=False)
    bb = nc.declare_dram_parameter("bb", [P, F], F32, isOutput=False)
    out = nc.declare_dram_parameter("o", [256, F], F32, isOutput=True)

    NT = S // P           # 8 s2 tiles
    NC_ = 2               # s1 chunks of 512

    import contextlib
    with tile.TileContext(nc) as tc:
        loop_ctx = tc.For_i(0, n_iter, 1, hint_engines=(
            mybir.EngineType.PE, mybir.EngineType.DVE, mybir.EngineType.Activation,
            mybir.EngineType.SP, mybir.EngineType.Pool,
        )) if n_iter > 1 else contextlib.nullcontext()
        with loop_ctx, \
             tc.tile_pool(name="cst", bufs=2) as cst, \
             tc.tile_pool(name="exw", bufs=4) as exw, \
             tc.tile_pool(name="wk", bufs=3) as wkp, \
             tc.tile_pool(name="yt", bufs=2) as ytp, \
             tc.tile_pool(name="pk", bufs=3, space="PSUM") as pkp, \
             tc.tile_pool(name="xps", bufs=2, space="PSUM") as xps:

            qkt_sb = cst.tile([P, 16, 2, S] if FP8 else [P, 16, S], QDT)
            va_sb = cst.tile([P, NT, 528], F16)
            wt_sb = cst.tile([P, F // P, F], F16)
            trid_sb = cst.tile([P, 2 * P], F16)
            bb_sb = cst.tile([P, F], F32)
            # issue input loads from the Pool (SWDGE) queue: SP.SEQ is
            # in-order and head-of-line blocked behind the previous
            # iteration's scatter/output DMAs, which would serialize the
            # reload against compute
            nc.gpsimd.dma_start(qkt_sb[:], qkt[:])
            nc.gpsimd.dma_start(va_sb[:], va[:])
            nc.gpsimd.dma_start(wt_sb[:], wt[:])
            nc.gpsimd.dma_start(trid_sb[:], trid[:])
            nc.gpsimd.dma_start(bb_sb[:], bb[:])

            yt_sb = ytp.tile([P, F // P, 256], F16)

            if causal:
                waves = WAVES
            else:
                # non-causal: 16 full slabs, 512-col pieces, waves of 2
                slabs = [(c, t, 0, 512, 512 * (i % 2), False)
                         for i, (c, t) in enumerate(
                             (c, t) for c in range(NC_) for t in range(NT))]
                waves = [slabs[i:i + 2] for i in range(0, 16, 2)]
            NW = len(waves)
            wave_list = [(g, wi) for g in range(NG) for wi in range(NW)]
            n_mm = sum(len(wv) for wv in waves)

            if FP8:
                # DoubleRow: [8 partitions, 2, cols]; head row r = p + 8*i
                def q_l(g, sl):
                    return qkt_sb[64 * (g % 2):64 * (g % 2) + 8, g // 2, :, sl]

                def k_l(g, sl):
                    return qkt_sb[64 * (g % 2):64 * (g % 2) + 8,
                                  8 + g // 2, :, sl]
            else:
                def q_l(g, sl):
                    return qkt_sb[64 * (g % 2):64 * (g % 2) + 32, g // 2, sl]

                def k_l(g, sl):
                    return qkt_sb[64 * (g % 2):64 * (g % 2) + 32,
                                  8 + g // 2, sl]

            def emit_scores(g, wi):
                """Score matmuls (with matmul'd causal tri-mask pre-bias) +
                one exp for wave wi of g."""
                wv = waves[wi]
                wlen = max(off + w for _, _, _, w, off, _ in wv)
                pkt = pkp.tile([P, 1024], F32, tag="pk", name=f"pk_{g}_{wi}")
                for c, t, a1, w, off, tri in wv:
                    if tri:
                        # causal triangle as a matmul: PSUM <- -1e4*[j<p],
                        # then the score matmul accumulates on top.
                        nc.tensor.matmul(
                            pkt[:, off:off + P], trid_sb[:, 0:P],
                            trid_sb[:, P:2 * P], start=True, stop=False, skip_group_check=True)
                        nc.tensor.matmul(
                            pkt[:, off:off + P],
                            k_l(g, slice(t * P, (t + 1) * P)),
                            q_l(g, slice(512 * c + a1, 512 * c + a1 + P)),
                            start=False, stop=True, perf_mode=DR, skip_group_check=True)
                        if w > P:
                            nc.tensor.matmul(
                                pkt[:, off + P:off + w],
                                k_l(g, slice(t * P, (t + 1) * P)),
                                q_l(g, slice(512 * c + a1 + P, 512 * (c + 1))),
                                start=True, stop=True, perf_mode=DR, skip_group_check=True)
                    else:
                        nc.tensor.matmul(
                            pkt[:, off:off + w],
                            k_l(g, slice(t * P, (t + 1) * P)),
                            q_l(g, slice(512 * c + a1, 512 * (c + 1))),
                            start=True, stop=True, perf_mode=DR, skip_group_check=True)
                ew = exw.tile([P, 1024], F16, tag="ew", name=f"ew_{g}_{wi}")
                nc.scalar.activation(ew[:, 0:wlen], pkt[:, 0:wlen], AF.Exp)
                return ew

            xt_of = {}
            # per-chunk xmm counts for start/stop flags
            n_mm_c = [sum(1 for wv in waves for s in wv if s[0] == c)
                      for c in range(NC_)]

            def emit_xmms(g, wi, ew):
                """x^T accumulation for wave wi of group g (lag pipelined).
                The 33-col stationary is [features(16) | zeros(16) | ones]:
                chunk 0 lands at xt partitions [0:33] (denominator row 32),
                chunk 1 at [64:97] (denominator row 96) -- feature reads at
                base 0/64 and denom reads at base 32/96 keep every engine AP
                32-partition aligned."""
                if wi == 0:
                    xt_of[g] = [xps.tile([P, 512], F32, tag="xt",
                                         name=f"xt_{g}"), [0] * NC_]
                xt, cnt = xt_of[g]
                for c, t, a1, w, off, tri in waves[wi]:
                    fo = 64 * c
                    nc.tensor.matmul(
                        xt[fo:fo + 33, a1:a1 + w] if causal
                        else xt[fo:fo + 33, 0:512],
                        va_sb[:, t, 33 * g:33 * g + 33], ew[:, off:off + w],
                        start=(cnt[c] == 0), stop=(cnt[c] == n_mm_c[c] - 1),
                        skip_group_check=True)
                    cnt[c] += 1

            def emit_normalize(g):
                """softmax-normalize + scatter both chunks of group g."""
                xt, _ = xt_of.pop(g)
                for c in range(NC_):
                    fo = 64 * c     # feature/denom partition offset in xt
                    recip = wkp.tile([1, 512], F16, tag="recip",
                                     name=f"rc_{g}_{c}")
                    with nc.allow_low_precision(reason="fp16 softmax recip"):
                        nc.vector.reciprocal(recip[:], xt[fo + 32:fo + 33, :])
                    recipb = wkp.tile([16, 512], F16, tag="recipb",
                                      name=f"rb_{g}_{c}")
                    nc.gpsimd.partition_broadcast(recipb[:], recip[:])
                    xs = wkp.tile([16, 4, 128], F16, tag="xs",
                                  name=f"xs_{g}_{c}")
                    # one strided op m-deinterleaves: out (m, r) <- col 4r+m
                    xt_v = xt[fo:fo + 16, :].rearrange("p (r m) -> p m r", m=4)
                    rb_v = recipb[:, :].rearrange("p (r m) -> p m r", m=4)
                    nc.vector.tensor_tensor(
                        out=xs[:], in0=xt_v, in1=rb_v, op=MUL)
                    g0 = g % 2
                    nc.sync.dma_start(
                        out=yt_sb[64 * g0:64 * g0 + 64, g // 2,
                                  128 * c:128 * (c + 1)],
                        in_=xs[:])

            # software pipeline: scores(w) || xmms(w-LAG), normalize at
            # group boundaries
            LAG = 3
            ew_of = {}
            for w, (g, wi) in enumerate(wave_list):
                ew_of[(g, wi)] = emit_scores(g, wi)
                if w >= LAG:
                    gp, wip = wave_list[w - LAG]
                    emit_xmms(gp, wip, ew_of.pop((gp, wip)))
                    if wip == NW - 1:
                        emit_normalize(gp)
            for w in range(len(wave_list) - LAG, len(wave_list)):
                gp, wip = wave_list[w]
                emit_xmms(gp, wip, ew_of.pop((gp, wip)))
                if wip == NW - 1:
                    emit_normalize(gp)

            # output linear: out[r, o] = sum_f yT[f, r] * wt[f, o] + b[o]
            for r2 in range(2):
                for oc in range(2):
                    ps_t = xps.tile([P, 512], F32, tag="xt")
                    ps = ps_t[:]
                    for ft_i in range(F // P):
                        nc.tensor.matmul(
                            ps, yt_sb[:, ft_i, r2 * P:(r2 + 1) * P],
                            wt_sb[:, ft_i, oc * 512:(oc + 1) * 512],
                            start=(ft_i == 0), stop=(ft_i == F // P - 1))
                    ot = wkp.tile([P, 512], F32, tag="ot")
                    nc.vector.tensor_tensor(
                        out=ot[:], in0=ps, in1=bb_sb[:, oc * 512:(oc + 1) * 512],
                        op=ADD)
                    nc.sync.dma_start(
                        out=out[r2 * P:(r2 + 1) * P, oc * 512:(oc + 1) * 512],
                        in_=ot[:])
    nc.compile()
    return nc


_NC_CACHE = {}


def _get_nc(causal: bool, n_iter: int = 1):
    key = (causal, n_iter)
    if key not in _NC_CACHE:
        _NC_CACHE[key] = _build(causal, n_iter)
    return _NC_CACHE[key]


def _shard_inputs(q, k, v, W_out, b_out):
    """Build the 8 per-core input maps."""
    wtT = np.ascontiguousarray(W_out.T).astype(np.float32)   # [f_in, o]
    wtp = np.empty((P, F // P, F), np.float16)
    for blk in range(F // P):
        for p in range(P):
            wtp[p, blk, :] = wtT[_perm_feature(p, blk), :]
    wtp = wtp.reshape(P, (F // P) * F)

    ki, pi = np.mgrid[0:P, 0:P]
    tridv = np.zeros((P, 2 * P), np.float16)
    tridv[:, 0:P] = np.where(ki < pi, -1e4, 0.0)      # strict lower tri
    tridv[:, P:2 * P] = np.where(ki == pi, 1.0, 0.0)  # identity
    bbv = np.broadcast_to(b_out.astype(np.float32), (P, F)).copy()

    if FP8:
        import concourse.mybir as mybir
        qdt = mybir.dt.np(mybir.dt.float8e4)

    in_maps = []
    for core in range(NCORES):
        _, _, b, js = _core_groups(core)
        cols = np.stack([j * H + np.arange(H) for j in js])    # [16, 16]
        if FP8:
            qkt = np.zeros((P, 16, 2, S), qdt)
        else:
            qkt = np.zeros((P, 16, S), np.float16)
        vav = np.zeros((P, NT_ := S // P, 528), np.float16)
        for g in range(NG):
            po, pb = 64 * (g % 2), g // 2
            qg = (0.25 * q[b][:, cols[g]].T)                   # [16, S]
            kg = k[b][:, cols[g]].T
            if FP8:
                # DoubleRow packing: head row r = p + 8*i
                qkt[po:po + 8, pb, 0, :] = qg[0:8].astype(qdt)
                qkt[po:po + 8, pb, 1, :] = qg[8:16].astype(qdt)
                qkt[po:po + 8, 8 + pb, 0, :] = kg[0:8].astype(qdt)
                qkt[po:po + 8, 8 + pb, 1, :] = kg[8:16].astype(qdt)
            else:
                qkt[po:po + 16, pb, :] = qg.astype(np.float16)
                qkt[po:po + 16, 8 + pb, :] = kg.astype(np.float16)
            vg = v[b][:, cols[g]].astype(np.float16)           # [S, 16]
            vg = vg.reshape(NT_, P, H).transpose(1, 0, 2)      # [P, NT, 16]
            vav[:, :, 33 * g:33 * g + 16] = vg
            vav[:, :, 33 * g + 32] = 1.0
        in_maps.append({
            "qkt": qkt.reshape(P, (32 if FP8 else 16) * S),
            "va": vav.reshape(P, (S // P) * 528),
            "wt": wtp,
            "trid": tridv,
            "bb": bbv,
        })
    return in_maps


def _unshard(outs):
    full = np.empty((B, S, F), np.float32)
    for c in range(NCORES):
        b2, qq, _, _ = _core_groups(c)
        full[b2, 256 * qq:256 * (qq + 1), :] = outs[c]
    return full


def _numpy_core(in_map, causal=True):
    """Numpy emulation of the device program (host-logic validation)."""
    if FP8:
        qkt = in_map["qkt"].reshape(P, 16, 2, S).astype(np.float32)
    else:
        qkt = in_map["qkt"].reshape(P, 16, S).astype(np.float32)
    va = in_map["va"].reshape(P, S // P, 528).astype(np.float32)
    wtm = in_map["wt"].reshape(P, F // P, F).astype(np.float32)
    bbv = in_map["bb"]
    ytv = np.zeros((P, F // P, 256), np.float32)
    for g in range(NG):
        po, pb = 64 * (g % 2), g // 2
        if FP8:
            qt = qkt[po:po + 8, pb].transpose(1, 0, 2).reshape(16, S)
            kt = qkt[po:po + 8, 8 + pb].transpose(1, 0, 2).reshape(16, S)
        else:
            qt = qkt[po:po + 16, pb, :]   # [16, S], pre-scaled
            kt = qkt[po:po + 16, 8 + pb, :]
        sc = kt.T @ qt                    # [s2, s1]
        if causal:
            s2i, s1i = np.mgrid[0:S, 0:S]
            sc = np.where(s1i >= s2i, sc, -1e9)
        e = np.exp(sc).astype(np.float16).astype(np.float32)
        if causal:
            e = np.where(s1i >= s2i, e, 0.0).astype(np.float32)
        # x accumulation with augmented va (feature rows + ones row at 32)
        vg = np.zeros((S, 33), np.float32)
        for t in range(S // P):
            vg[t * P:(t + 1) * P, :] = va[:, t, 33 * g:33 * g + 33]
        xt = vg.T @ e                     # [33, s1]
        recip = (1.0 / xt[32]).astype(np.float16).astype(np.float32)
        xs = (xt[0:16] * recip[None, :]).astype(np.float16).astype(np.float32)
        g0 = g % 2
        for c in range(2):
            for h in range(16):
                for m in range(4):
                    p = 64 * g0 + 4 * h + m
                    ytv[p, g // 2, 128 * c:128 * (c + 1)] = \
                        xs[h, 512 * c + m:512 * (c + 1):4]
    yt2 = ytv.reshape(P, (F // P) * 256)
    o = np.zeros((256, F), np.float32)
    for r2 in range(2):
        for oc in range(2):
            acc = np.zeros((P, 512), np.float32)
            for ft_i in range(F // P):
                acc += ytv[:, ft_i, r2 * P:(r2 + 1) * P].T @ \
                    wtm[:, ft_i, oc * 512:(oc + 1) * 512]
            o[r2 * P:(r2 + 1) * P, oc * 512:(oc + 1) * 512] = \
                acc + bbv[0][None, oc * 512:(oc + 1) * 512]
    return o.astype(np.float32)


def kernel(q, k, v, W_out, b_out, apply_mask, _mock=False):
    q = np.asarray(q, np.float32)
    k = np.asarray(k, np.float32)
    v = np.asarray(v, np.float32)
    W_out = np.asarray(W_out, np.float32)
    b_out = np.asarray(b_out, np.float32)
    causal = bool(int(np.asarray(apply_mask)))
    in_maps = _shard_inputs(q, k, v, W_out, b_out)
    if _mock:
        outs = [_numpy_core(m, causal) for m in in_maps]
        return _unshard(outs)
    from concourse.bass_utils import run_bass_kernel_spmd
    nc = _get_nc(causal)
    res = run_bass_kernel_spmd(nc, in_maps, core_ids=list(range(NCORES)))
    return _unshard([r["o"] for r in res.results])
